# revision 3
# baseline (speedup 1.0000x reference)
"""Additive attention, query-position-sharded Bass kernel for 8 TRN2 cores.

Sharding: each core owns 1/8 of every batch's valid q-range
(qc_b = ceil(vl_b/8) columns, identical across cores -> near-perfect SPMD
balance at ~sum(vl)/8 columns x 256 k per core). Softmax over q is split
across cores: each core emits unnormalized partial attn@value and a
partial denominator; the host sums partials and divides (exact).

Per-core pipeline (per q column j of batch b):
  DVE:  sum[:, slot, :] = kT_b(bf16) + qT[:, j]   (tensor_scalar add, 256 free)
  ACT:  tanh in-place over tapered chunks (one big ACTIVATE per chunk)
  PE:   per column x 2 k-blocks: score col = feat^T @ wv into PSUM
  ACT:  exp over grouped batches' score tiles
  PE:   attn transpose, attn^T @ value (AV), attn^T @ colmask (den)
Masked (padded) columns are neutralized by host-zeroed value rows (AV)
and the 0/1 column mask (den) -- no on-device masking.
"""

import numpy as np
import ml_dtypes

import concourse.bass as bass
import concourse.bacc as bacc
import concourse.tile as tile
from concourse import mybir
from concourse.bass_utils import run_bass_kernel_spmd

B = 16
NK = 256
NQ = 256
DK = 256
DV = 256
H = 128
P = 128
NCORES = 8
CH = 64          # max q-columns per tanh chunk
QCMAX = 32       # max cols per (batch, core)
GROUP_F32 = 128  # f32 columns per score-psum group tile

F32 = mybir.dt.float32
BF16 = mybir.dt.bfloat16
TANH = mybir.ActivationFunctionType.Tanh
EXP = mybir.ActivationFunctionType.Exp

BF = ml_dtypes.bfloat16
USE_XBAR = False

_CACHE = {}


def _plan(qc):
    desc = sorted(range(B), key=lambda b: (-qc[b], b))
    # interleave big/small so score groups complete evenly through the
    # stream; ends on the smallest batch for a short drain
    order = []
    for i in range(B // 2):
        order += [desc[i], desc[i + B // 2]]
    cols = []  # (batch, j, qT column index)
    off = 0
    for b in order:
        for j in range(qc[b]):
            cols.append((b, j, off + j))
        off += qc[b]
    ncols = off
    # score groups: consecutive batches, sum(2*qc) <= GROUP_F32; last four
    # batches ride solo so the drain pipeline is fine-grained
    groups = []
    cur, cur_sz = [], 0
    for b in order[:-2]:
        if cur and cur_sz + 2 * qc[b] > GROUP_F32:
            groups.append(cur)
            cur, cur_sz = [], 0
        cur.append(b)
        cur_sz += 2 * qc[b]
    if cur:
        groups.append(cur)
    for b in order[-2:]:
        groups.append([b])
    gidx, gbase = {}, {}
    for gi, g in enumerate(groups):
        boff = 0
        for b in g:
            gidx[b] = gi
            gbase[b] = boff
            boff += 2 * qc[b]
    return order, cols, ncols, groups, gidx, gbase


def _chunk_sizes(n):
    sizes = []
    for r in (8, 12, 16, 20, 28):
        if sum(sizes) + r <= n:
            sizes.append(r)
    while n - sum(sizes) > 48:
        sizes.append(32)
    rem = n - sum(sizes)
    if rem > 8:
        sizes += [rem - 8, 8]
    elif rem > 0:
        sizes.append(rem)
    return sizes


def _build(qc):
    order, cols, ncols, groups, gidx, gbase = _plan(qc)

    nc = bacc.Bacc("TRN2", target_bir_lowering=False, debug=False,
                   num_devices=NCORES)

    keyT_d = nc.dram_tensor("keyT", [B, P, 2, NK], BF16, kind="ExternalInput")
    qryT_d = nc.dram_tensor("qryT", [P, 2, ncols], BF16, kind="ExternalInput")
    val_d = nc.dram_tensor("val", [QCMAX, B, DV], BF16, kind="ExternalInput")
    mask_d = nc.dram_tensor("maskP", [QCMAX, B], BF16, kind="ExternalInput")
    wk_d = nc.dram_tensor("Wk", [P, 2, H], BF16, kind="ExternalInput")
    wq_d = nc.dram_tensor("Wq", [P, 2, H], BF16, kind="ExternalInput")
    wv_d = nc.dram_tensor("wv", [H, 1], BF16, kind="ExternalInput")
    id_d = nc.dram_tensor("ident", [P, P], BF16, kind="ExternalInput")
    av_d = nc.dram_tensor("av", [B, P, 2, DV], BF16, kind="ExternalOutput")
    den_d = nc.dram_tensor("den", [P, 2 * B], F32, kind="ExternalOutput")

    with tile.TileContext(nc) as tc:
        with (
            tc.tile_pool(name="const", bufs=1) as const,
            tc.tile_pool(name="big", bufs=1) as big,
            tc.tile_pool(name="kin", bufs=2) as kin,
            tc.tile_pool(name="sumr", bufs=3) as sumr,
            tc.tile_pool(name="attnp", bufs=2) as attnp,
            tc.tile_pool(name="atp", bufs=4) as atp,
            tc.tile_pool(name="ps_proj", bufs=1, space="PSUM") as ps_proj,
            tc.tile_pool(name="ps_sc", bufs=2, space="PSUM") as ps_sc,
            tc.tile_pool(name="ps_av", bufs=2, space="PSUM") as ps_av,
            tc.tile_pool(name="ps_den", bufs=1, space="PSUM") as ps_den,
        ):
            wk_sb = const.tile([P, 2, H], BF16)
            wq_sb = const.tile([P, 2, H], BF16)
            wv_sb = const.tile([H, 1], BF16)
            id_sb = const.tile([P, P], BF16)
            qryT_sb = const.tile([P, 2, ncols], BF16)
            val_sb = const.tile([QCMAX, B, DV], BF16)
            mask_sb = const.tile([QCMAX, B], BF16)

            kT_sb = big.tile([P, B, NK], BF16)   # h-major projected keys
            qT_sb = big.tile([P, ncols], F32)    # h-major projected queries

            # ---- input DMAs (first-need first) ----
            keyT_tiles = {}
            _kq = [0]

            def load_key(b):
                t = kin.tile([P, 2, NK], BF16, name=f"keyT{b}", tag="keyT")
                keyT_tiles[b] = t
                eng = nc.sync if (_kq[0] % 2 == 0) else nc.gpsimd
                _kq[0] += 1
                eng.dma_start(out=t, in_=keyT_d[b, :, :, :])

            nc.sync.dma_start(out=qryT_sb[:, :, :32], in_=qryT_d[:, :, :32])
            nc.gpsimd.dma_start(out=wk_sb, in_=wk_d[:, :, :])
            load_key(order[0])        # sync
            nc.gpsimd.dma_start(out=wq_sb, in_=wq_d[:, :, :])
            load_key(order[1])        # gpsimd
            nc.gpsimd.dma_start(out=wv_sb, in_=wv_d[:, :])
            nc.gpsimd.dma_start(out=id_sb, in_=id_d[:, :])
            nc.gpsimd.dma_start(out=mask_sb, in_=mask_d[:, :])
            nc.gpsimd.dma_start(out=val_sb, in_=val_d[:, :, :])

            projected = set()

            def kproj(b):
                projected.add(b)
                kp = ps_proj.tile([P, NK], F32, name=f"kproj{b}", tag="proj")
                for dk in range(2):
                    nc.tensor.matmul(kp, wk_sb[:, dk, :],
                                     keyT_tiles[b][:, dk, :],
                                     start=(dk == 0), stop=(dk == 1))
                if 1 <= order.index(b) < 6:
                    nc.scalar.copy(kT_sb[:, b, :], kp)
                else:
                    nc.vector.tensor_copy(kT_sb[:, b, :], kp)

            kproj(order[0])

            nc.sync.dma_start(out=qryT_sb[:, :, 32:], in_=qryT_d[:, :, 32:])

            # ---- q projection: qT[h, c] = sum_dk Wq[dk,h] * qryT[dk,c] ----
            qp = ps_proj.tile([P, ncols], F32, name="qproj", tag="proj")
            for dk in range(2):
                nc.tensor.matmul(qp[:, :32], wq_sb[:, dk, :],
                                 qryT_sb[:, dk, :32],
                                 start=(dk == 0), stop=(dk == 1))
            nc.vector.tensor_copy(qT_sb[:, :32], qp[:, :32])
            for dk in range(2):
                nc.tensor.matmul(qp[:, 32:], wq_sb[:, dk, :],
                                 qryT_sb[:, dk, 32:],
                                 start=(dk == 0), stop=(dk == 1))
            nc.vector.tensor_copy(qT_sb[:, 32:], qp[:, 32:])

            kproj(order[1])

            # ---- score psum groups ----
            gtiles = [
                ps_sc.tile([P, GROUP_F32], F32, name=f"sg{gi}", tag="sg")
                for gi in range(len(groups))
            ]
            den_ps = ps_den.tile([P, 2 * B], F32)
            last_col = {}
            for ci_, (b, j, qi) in enumerate(cols):
                last_col[b] = ci_
            group_done_at = {}
            for gi, g in enumerate(groups):
                group_done_at[max(last_col[b] for b in g)] = gi

            deferred = []  # (due_chunk, closure), kept due-sorted
            cur_ci = [0]
            _avq = [0]

            def push(due, fn):
                deferred.append((due, fn))
                deferred.sort(key=lambda t: t[0])

            def drain(force=False):
                while deferred and (force or deferred[0][0] <= cur_ci[0]):
                    deferred.pop(0)[1]()
                    if not force:
                        break

            def emit_epilogue(gi, ci):
                g = groups[gi]
                gt = gtiles[gi]
                used = sum(2 * qc[b] for b in g)

                def stage_a():
                    attn_sb = attnp.tile([P, GROUP_F32], BF16,
                                         name=f"attn{gi}", tag="attn")
                    nc.scalar.activation(out=attn_sb[:, :used],
                                         in_=gt[:, :used], func=EXP)
                    for b in g:
                        n = qc[b]
                        tp = ps_av.tile([P, 2, P], BF16, name=f"tp{b}",
                                        tag="tp")
                        for kb in range(2):
                            sl = attn_sb[:, gbase[b] + kb * n:
                                         gbase[b] + (kb + 1) * n]
                            nc.tensor.transpose(tp[:n, kb, :], sl, id_sb)
                        push(ci + 3, stage_b1(b, tp))

                def stage_b1(b, tp):
                    def go():
                        n = qc[b]
                        at = atp.tile([P, 2, P], BF16, name=f"at{b}", tag="at")
                        nc.vector.tensor_copy(at[:n, :, :], tp[:n, :, :])
                        av = ps_av.tile([P, 2, DV], F32, name=f"av{b}",
                                        tag="av")
                        for kb in range(2):
                            nc.tensor.matmul(av[:, kb, :], at[:n, kb, :],
                                             val_sb[:, b, :][0:n, :],
                                             start=True, stop=True)
                            nc.tensor.matmul(
                                den_ps[:, 2 * b + kb: 2 * b + kb + 1],
                                at[:n, kb, :], mask_sb[:, b: b + 1][0:n, :],
                                start=True, stop=True)
                        def go2():
                            av_sb = atp.tile([P, 2, DV], BF16,
                                             name=f"avs{b}", tag="avs")
                            if gidx[b] >= len(groups) - 3:
                                nc.scalar.copy(av_sb, av)
                            else:
                                nc.vector.tensor_copy(av_sb, av)
                            eng = nc.sync if (_avq[0] % 2 == 0) else nc.gpsimd
                            _avq[0] += 1
                            eng.dma_start(out=av_d[b, :, :, :], in_=av_sb)
                        push(ci + 4, go2)
                    return go

                push(ci + 1, stage_a)

            # ---- main column stream ----
            sizes = _chunk_sizes(len(cols))
            starts = [sum(sizes[:i]) for i in range(len(sizes))]
            for ci, (c0, csz) in enumerate(zip(starts, sizes)):
                cur_ci[0] = ci
                chunk = cols[c0: c0 + csz]
                nxt = sizes[ci + 1] if ci + 1 < len(sizes) else 0
                horizon = {b for (b, j, qi) in
                           cols[c0: c0 + csz + nxt] if j == 0}
                for b in order:
                    if b in horizon and b not in keyT_tiles:
                        load_key(b)
                for b in order:
                    if b in horizon and b not in projected:
                        kproj(b)

                st = sumr.tile([P, CH, NK], BF16, name=f"sum{ci}", tag="sum")
                for si, (b, j, qi) in enumerate(chunk):
                    nc.vector.tensor_scalar_add(
                        out=st[:, si, :], in0=kT_sb[:, b, :],
                        scalar1=qT_sb[:, qi: qi + 1])
                    if si % 2 == 1 and 3 * si >= 2 * csz:
                        drain()
                g = len(chunk)
                nc.scalar.activation(out=st[:, :g, :], in_=st[:, :g, :],
                                     func=TANH)
                for si, (b, j, qi) in enumerate(chunk):
                    gt = gtiles[gidx[b]]
                    base = gbase[b]
                    n = qc[b]
                    for kb in range(2):
                        nc.tensor.matmul(
                            gt[:, base + kb * n + j: base + kb * n + j + 1],
                            st[:, si, kb * P: (kb + 1) * P], wv_sb,
                            start=True, stop=True)
                for done_at, gi in sorted(group_done_at.items()):
                    if c0 <= done_at <= c0 + csz - 1:
                        emit_epilogue(gi, ci)

            drain(force=True)
            den_sb = const.tile([P, 2 * B], F32)
            nc.scalar.copy(den_sb, den_ps)
            nc.sync.dma_start(out=den_d[:, :], in_=den_sb)

    nc.compile()
    return nc


def kernel(key, query, value, valid_lens, Wk, Wq, wv, _trace=False):
    key = np.asarray(key, dtype=np.float32)
    query = np.asarray(query, dtype=np.float32)
    value = np.asarray(value, dtype=np.float32)
    valid_lens = np.asarray(valid_lens)
    Wk = np.asarray(Wk, dtype=np.float32)
    Wq = np.asarray(Wq, dtype=np.float32)
    wv = np.asarray(wv, dtype=np.float32)

    vl = np.clip(valid_lens.astype(np.int64), 1, NQ)
    qc = [int(-(-v // NCORES)) for v in vl]
    qkey = tuple(qc)
    if qkey not in _CACHE:
        _CACHE[qkey] = _build(qc)
    nc = _CACHE[qkey]
    order, cols, ncols, groups, gidx, gbase = _plan(qc)

    keyT = np.ascontiguousarray(
        key.transpose(0, 2, 1).reshape(B, 2, P, NK).transpose(0, 2, 1, 3)
    ).astype(BF)
    wk_h = np.ascontiguousarray(Wk.reshape(2, P, H).transpose(1, 0, 2)).astype(BF)
    wq_h = np.ascontiguousarray(Wq.reshape(2, P, H).transpose(1, 0, 2)).astype(BF)
    wv_h = wv.reshape(H, 1).astype(BF)
    id_h = np.eye(P, dtype=np.float32).astype(BF)

    in_maps = []
    for c in range(NCORES):
        qryT = np.zeros((DK, ncols), dtype=np.float32)
        valp = np.zeros((QCMAX, B, DV), dtype=np.float32)
        maskp = np.zeros((QCMAX, B), dtype=np.float32)
        off = 0
        for b in order:
            n = qc[b]
            lo = c * n
            rows = query[b, lo: lo + n, :]          # (n, DK)
            qryT[:, off: off + n] = rows.T
            nvalid = int(np.clip(vl[b] - lo, 0, n))
            if nvalid > 0:
                valp[:nvalid, b, :] = value[b, lo: lo + nvalid, :]
                maskp[:nvalid, b] = 1.0
            off += n
        in_maps.append({
            "keyT": keyT,
            "qryT": np.ascontiguousarray(
                qryT.reshape(2, P, ncols).transpose(1, 0, 2)).astype(BF),
            "val": valp.astype(BF),
            "maskP": maskp.astype(BF),
            "Wk": wk_h,
            "Wq": wq_h,
            "wv": wv_h,
            "ident": id_h,
        })

    res = run_bass_kernel_spmd(nc, in_maps, core_ids=list(range(NCORES)),
                               trace=_trace)
    kernel.last_results = res

    av = np.zeros((B, P, 2, DV), dtype=np.float64)
    den = np.zeros((P, 2 * B), dtype=np.float64)
    for c in range(NCORES):
        av += np.asarray(res.results[c]["av"], dtype=np.float64)
        den += np.asarray(res.results[c]["den"], dtype=np.float64)
    out = np.empty((B, NK, DV), dtype=np.float32)
    for b in range(B):
        for kb in range(2):
            d = den[:, 2 * b + kb]              # (128,)
            out[b, kb * P: (kb + 1) * P, :] = (
                av[b, :, kb, :] / d[:, None]).astype(np.float32)
    return out


# revision 4
# speedup vs baseline: 1.1779x; 1.1779x over previous
"""Additive attention, query-position-sharded Bass kernel for 8 TRN2 cores.

Sharding: each core owns 1/8 of every batch's valid q-range
(qc_b = ceil(vl_b/8) columns, identical across cores -> near-perfect SPMD
balance at ~sum(vl)/8 columns x 256 k per core). Softmax over q is split
across cores: each core emits unnormalized partial attn@value and a
partial denominator; the host sums partials and divides (exact).

Per-core pipeline (per q column j of batch b):
  DVE:  sum[:, slot, :] = kT_b(bf16) + qT[:, j]   (tensor_scalar add, 256 free)
  ACT:  tanh in-place over tapered chunks (one big ACTIVATE per chunk)
  PE:   per column x 2 k-blocks: score col = feat^T @ wv into PSUM
  ACT:  exp over grouped batches' score tiles
  PE:   attn transpose, attn^T @ value (AV), attn^T @ colmask (den)
Masked (padded) columns are neutralized by host-zeroed value rows (AV)
and the 0/1 column mask (den) -- no on-device masking.
"""

import numpy as np
import ml_dtypes

import concourse.bass as bass
import concourse.bacc as bacc
import concourse.tile as tile
from concourse import mybir
from concourse.bass_utils import run_bass_kernel_spmd

B = 16
NK = 256
NQ = 256
DK = 256
DV = 256
H = 128
P = 128
NCORES = 8
CH = 64          # max q-columns per tanh chunk
QCMAX = 32       # max cols per (batch, core)
GROUP_F32 = 128  # f32 columns per score-psum group tile

F32 = mybir.dt.float32
BF16 = mybir.dt.bfloat16
TANH = mybir.ActivationFunctionType.Tanh
EXP = mybir.ActivationFunctionType.Exp

BF = ml_dtypes.bfloat16
USE_XBAR = False

_CACHE = {}


def _plan(qc):
    desc = sorted(range(B), key=lambda b: (-qc[b], b))
    # interleave big/small so score groups complete evenly through the
    # stream; ends on the smallest batch for a short drain
    order = []
    for i in range(B // 2):
        order += [desc[i], desc[i + B // 2]]
    cols = []  # (batch, j, qT column index)
    off = 0
    for b in order:
        for j in range(qc[b]):
            cols.append((b, j, off + j))
        off += qc[b]
    ncols = off
    # score groups: consecutive batches, sum(2*qc) <= GROUP_F32; last four
    # batches ride solo so the drain pipeline is fine-grained
    groups = []
    cur, cur_sz = [], 0
    for b in order[:-2]:
        if cur and cur_sz + 2 * qc[b] > GROUP_F32:
            groups.append(cur)
            cur, cur_sz = [], 0
        cur.append(b)
        cur_sz += 2 * qc[b]
    if cur:
        groups.append(cur)
    for b in order[-2:]:
        groups.append([b])
    gidx, gbase = {}, {}
    for gi, g in enumerate(groups):
        boff = 0
        for b in g:
            gidx[b] = gi
            gbase[b] = boff
            boff += 2 * qc[b]
    return order, cols, ncols, groups, gidx, gbase


def _chunk_sizes(n):
    sizes = []
    for r in (8, 12, 16, 20, 28):
        if sum(sizes) + r <= n:
            sizes.append(r)
    while n - sum(sizes) > 48:
        sizes.append(32)
    rem = n - sum(sizes)
    if rem > 8:
        sizes += [rem - 8, 8]
    elif rem > 0:
        sizes.append(rem)
    return sizes


def _build(qc):
    order, cols, ncols, groups, gidx, gbase = _plan(qc)

    nc = bacc.Bacc("TRN2", target_bir_lowering=False, debug=False,
                   num_devices=NCORES)

    keyT_d = nc.dram_tensor("keyT", [B, P, 2, NK], BF16, kind="ExternalInput")
    qryT_d = nc.dram_tensor("qryT", [P, 2, ncols], BF16, kind="ExternalInput")
    val_d = nc.dram_tensor("val", [QCMAX, B, DV], BF16, kind="ExternalInput")
    mask_d = nc.dram_tensor("maskP", [QCMAX, B], BF16, kind="ExternalInput")
    wk_d = nc.dram_tensor("Wk", [P, 2, H], BF16, kind="ExternalInput")
    wq_d = nc.dram_tensor("Wq", [P, 2, H], BF16, kind="ExternalInput")
    wv_d = nc.dram_tensor("wv", [H, 1], BF16, kind="ExternalInput")
    id_d = nc.dram_tensor("ident", [P, P], BF16, kind="ExternalInput")
    av_d = nc.dram_tensor("av", [B, P, 2, DV], BF16, kind="ExternalOutput")
    den_d = nc.dram_tensor("den", [P, 2 * B], F32, kind="ExternalOutput")

    with tile.TileContext(nc) as tc:
        with (
            tc.tile_pool(name="const", bufs=1) as const,
            tc.tile_pool(name="big", bufs=1) as big,
            tc.tile_pool(name="kin", bufs=2) as kin,
            tc.tile_pool(name="sumr", bufs=3) as sumr,
            tc.tile_pool(name="attnp", bufs=2) as attnp,
            tc.tile_pool(name="atp", bufs=4) as atp,
            tc.tile_pool(name="ps_proj", bufs=1, space="PSUM") as ps_proj,
            tc.tile_pool(name="ps_sc", bufs=2, space="PSUM") as ps_sc,
            tc.tile_pool(name="ps_av", bufs=2, space="PSUM") as ps_av,
            tc.tile_pool(name="ps_den", bufs=1, space="PSUM") as ps_den,
        ):
            wk_sb = const.tile([P, 2, H], BF16)
            wq_sb = const.tile([P, 2, H], BF16)
            wv_sb = const.tile([H, 1], BF16)
            id_sb = const.tile([P, P], BF16)
            qryT_sb = const.tile([P, 2, ncols], BF16)
            val_sb = const.tile([QCMAX, B, DV], BF16)
            mask_sb = const.tile([QCMAX, B], BF16)

            kT_sb = big.tile([P, B, NK], BF16)   # h-major projected keys
            qT_sb = big.tile([P, ncols], F32)    # h-major projected queries

            # ---- input DMAs (first-need first) ----
            keyT_tiles = {}
            _kq = [0]

            def load_key(b):
                t = kin.tile([P, 2, NK], BF16, name=f"keyT{b}", tag="keyT")
                keyT_tiles[b] = t
                eng = nc.sync if (_kq[0] % 2 == 0) else nc.gpsimd
                _kq[0] += 1
                eng.dma_start(out=t, in_=keyT_d[b, :, :, :])

            nc.sync.dma_start(out=qryT_sb[:, :, :32], in_=qryT_d[:, :, :32])
            nc.gpsimd.dma_start(out=wk_sb, in_=wk_d[:, :, :])
            load_key(order[0])        # sync
            nc.gpsimd.dma_start(out=wq_sb, in_=wq_d[:, :, :])
            load_key(order[1])        # gpsimd
            nc.gpsimd.dma_start(out=wv_sb, in_=wv_d[:, :])
            nc.gpsimd.dma_start(out=id_sb, in_=id_d[:, :])
            nc.gpsimd.dma_start(out=mask_sb, in_=mask_d[:, :])
            nc.gpsimd.dma_start(out=val_sb, in_=val_d[:, :, :])

            projected = set()

            def kproj(b):
                projected.add(b)
                kp = ps_proj.tile([P, NK], F32, name=f"kproj{b}", tag="proj")
                for dk in range(2):
                    nc.tensor.matmul(kp, wk_sb[:, dk, :],
                                     keyT_tiles[b][:, dk, :],
                                     start=(dk == 0), stop=(dk == 1))
                if 1 <= order.index(b) < 9:
                    nc.scalar.copy(kT_sb[:, b, :], kp)
                else:
                    nc.vector.tensor_copy(kT_sb[:, b, :], kp)

            kproj(order[0])

            nc.sync.dma_start(out=qryT_sb[:, :, 32:], in_=qryT_d[:, :, 32:])

            # ---- q projection: qT[h, c] = sum_dk Wq[dk,h] * qryT[dk,c] ----
            qp = ps_proj.tile([P, ncols], F32, name="qproj", tag="proj")
            for dk in range(2):
                nc.tensor.matmul(qp[:, :32], wq_sb[:, dk, :],
                                 qryT_sb[:, dk, :32],
                                 start=(dk == 0), stop=(dk == 1))
            nc.vector.tensor_copy(qT_sb[:, :32], qp[:, :32])
            for dk in range(2):
                nc.tensor.matmul(qp[:, 32:], wq_sb[:, dk, :],
                                 qryT_sb[:, dk, 32:],
                                 start=(dk == 0), stop=(dk == 1))
            nc.vector.tensor_copy(qT_sb[:, 32:], qp[:, 32:])

            kproj(order[1])

            # ---- score psum groups ----
            gtiles = [
                ps_sc.tile([P, GROUP_F32], F32, name=f"sg{gi}", tag="sg")
                for gi in range(len(groups))
            ]
            den_ps = ps_den.tile([P, 2 * B], F32)
            last_col = {}
            for ci_, (b, j, qi) in enumerate(cols):
                last_col[b] = ci_
            group_done_at = {}
            for gi, g in enumerate(groups):
                group_done_at[max(last_col[b] for b in g)] = gi

            deferred = []  # (due_chunk, closure), kept due-sorted
            cur_ci = [0]
            _avq = [0]

            def push(due, fn):
                deferred.append((due, fn))
                deferred.sort(key=lambda t: t[0])

            def drain(force=False):
                while deferred and (force or deferred[0][0] <= cur_ci[0]):
                    deferred.pop(0)[1]()
                    if not force:
                        break

            def emit_epilogue(gi, ci):
                g = groups[gi]
                gt = gtiles[gi]
                used = sum(2 * qc[b] for b in g)

                def stage_a():
                    attn_sb = attnp.tile([P, GROUP_F32], BF16,
                                         name=f"attn{gi}", tag="attn")
                    nc.scalar.activation(out=attn_sb[:, :used],
                                         in_=gt[:, :used], func=EXP)
                    for b in g:
                        n = qc[b]
                        tp = ps_av.tile([P, 2, P], BF16, name=f"tp{b}",
                                        tag="tp")
                        for kb in range(2):
                            sl = attn_sb[:, gbase[b] + kb * n:
                                         gbase[b] + (kb + 1) * n]
                            nc.tensor.transpose(tp[:n, kb, :], sl, id_sb)
                        push(ci + 3, stage_b1(b, tp))

                def stage_b1(b, tp):
                    def go():
                        n = qc[b]
                        at = atp.tile([P, 2, P], BF16, name=f"at{b}", tag="at")
                        nc.vector.tensor_copy(at[:n, :, :], tp[:n, :, :])
                        av = ps_av.tile([P, 2, DV], F32, name=f"av{b}",
                                        tag="av")
                        for kb in range(2):
                            nc.tensor.matmul(av[:, kb, :], at[:n, kb, :],
                                             val_sb[:, b, :][0:n, :],
                                             start=True, stop=True)
                            nc.tensor.matmul(
                                den_ps[:, 2 * b + kb: 2 * b + kb + 1],
                                at[:n, kb, :], mask_sb[:, b: b + 1][0:n, :],
                                start=True, stop=True)
                        def go2():
                            av_sb = atp.tile([P, 2, DV], BF16,
                                             name=f"avs{b}", tag="avs")
                            if gidx[b] >= len(groups) - 3:
                                nc.scalar.copy(av_sb, av)
                            else:
                                nc.vector.tensor_copy(av_sb, av)
                            eng = nc.sync if (_avq[0] % 2 == 0) else nc.gpsimd
                            _avq[0] += 1
                            eng.dma_start(out=av_d[b, :, :, :], in_=av_sb)
                        push(ci + 4, go2)
                    return go

                push(ci + 1, stage_a)

            # ---- main column stream ----
            sizes = _chunk_sizes(len(cols))
            starts = [sum(sizes[:i]) for i in range(len(sizes))]
            for ci, (c0, csz) in enumerate(zip(starts, sizes)):
                cur_ci[0] = ci
                chunk = cols[c0: c0 + csz]
                nxt = sizes[ci + 1] if ci + 1 < len(sizes) else 0
                horizon = {b for (b, j, qi) in
                           cols[c0: c0 + csz + nxt] if j == 0}
                for b in order:
                    if b in horizon and b not in keyT_tiles:
                        load_key(b)
                for b in order:
                    if b in horizon and b not in projected:
                        kproj(b)

                st = sumr.tile([P, CH, NK], BF16, name=f"sum{ci}", tag="sum")
                for si, (b, j, qi) in enumerate(chunk):
                    nc.vector.tensor_scalar_add(
                        out=st[:, si, :], in0=kT_sb[:, b, :],
                        scalar1=qT_sb[:, qi: qi + 1])
                    if si % 2 == 1 and 3 * si >= 2 * csz:
                        drain()
                g = len(chunk)
                nc.scalar.activation(out=st[:, :g, :], in_=st[:, :g, :],
                                     func=TANH)
                for si, (b, j, qi) in enumerate(chunk):
                    gt = gtiles[gidx[b]]
                    base = gbase[b]
                    n = qc[b]
                    for kb in range(2):
                        nc.tensor.matmul(
                            gt[:, base + kb * n + j: base + kb * n + j + 1],
                            st[:, si, kb * P: (kb + 1) * P], wv_sb,
                            start=True, stop=True)
                for done_at, gi in sorted(group_done_at.items()):
                    if c0 <= done_at <= c0 + csz - 1:
                        emit_epilogue(gi, ci)

            drain(force=True)
            den_sb = const.tile([P, 2 * B], F32)
            nc.scalar.copy(den_sb, den_ps)
            nc.sync.dma_start(out=den_d[:, :], in_=den_sb)

    nc.compile()
    return nc


def kernel(key, query, value, valid_lens, Wk, Wq, wv, _trace=False):
    key = np.asarray(key, dtype=np.float32)
    query = np.asarray(query, dtype=np.float32)
    value = np.asarray(value, dtype=np.float32)
    valid_lens = np.asarray(valid_lens)
    Wk = np.asarray(Wk, dtype=np.float32)
    Wq = np.asarray(Wq, dtype=np.float32)
    wv = np.asarray(wv, dtype=np.float32)

    vl = np.clip(valid_lens.astype(np.int64), 1, NQ)
    qc = [int(-(-v // NCORES)) for v in vl]
    qkey = tuple(qc)
    if qkey not in _CACHE:
        _CACHE[qkey] = _build(qc)
    nc = _CACHE[qkey]
    order, cols, ncols, groups, gidx, gbase = _plan(qc)

    keyT = np.ascontiguousarray(
        key.transpose(0, 2, 1).reshape(B, 2, P, NK).transpose(0, 2, 1, 3)
    ).astype(BF)
    wk_h = np.ascontiguousarray(Wk.reshape(2, P, H).transpose(1, 0, 2)).astype(BF)
    wq_h = np.ascontiguousarray(Wq.reshape(2, P, H).transpose(1, 0, 2)).astype(BF)
    wv_h = wv.reshape(H, 1).astype(BF)
    id_h = np.eye(P, dtype=np.float32).astype(BF)

    in_maps = []
    for c in range(NCORES):
        qryT = np.zeros((DK, ncols), dtype=np.float32)
        valp = np.zeros((QCMAX, B, DV), dtype=np.float32)
        maskp = np.zeros((QCMAX, B), dtype=np.float32)
        off = 0
        for b in order:
            n = qc[b]
            lo = c * n
            rows = query[b, lo: lo + n, :]          # (n, DK)
            qryT[:, off: off + n] = rows.T
            nvalid = int(np.clip(vl[b] - lo, 0, n))
            if nvalid > 0:
                valp[:nvalid, b, :] = value[b, lo: lo + nvalid, :]
                maskp[:nvalid, b] = 1.0
            off += n
        in_maps.append({
            "keyT": keyT,
            "qryT": np.ascontiguousarray(
                qryT.reshape(2, P, ncols).transpose(1, 0, 2)).astype(BF),
            "val": valp.astype(BF),
            "maskP": maskp.astype(BF),
            "Wk": wk_h,
            "Wq": wq_h,
            "wv": wv_h,
            "ident": id_h,
        })

    res = run_bass_kernel_spmd(nc, in_maps, core_ids=list(range(NCORES)),
                               trace=_trace)
    kernel.last_results = res

    av = np.zeros((B, P, 2, DV), dtype=np.float64)
    den = np.zeros((P, 2 * B), dtype=np.float64)
    for c in range(NCORES):
        av += np.asarray(res.results[c]["av"], dtype=np.float64)
        den += np.asarray(res.results[c]["den"], dtype=np.float64)
    out = np.empty((B, NK, DV), dtype=np.float32)
    for b in range(B):
        for kb in range(2):
            d = den[:, 2 * b + kb]              # (128,)
            out[b, kb * P: (kb + 1) * P, :] = (
                av[b, :, kb, :] / d[:, None]).astype(np.float32)
    return out
